# revision 8
# baseline (speedup 1.0000x reference)
"""Trainium2 Bass kernel for nn_BitDiscriminatorP (BitNet conv discriminator, period 2).

Strategy:
- Pure data parallelism: batch 32 sharded 4-per-core across 8 NeuronCores.
- All convs are 1-D convs over H (the period dim has kernel 1) -> expressed as
  accumulating matmuls on the tensor engine with stride-3 access patterns, with
  the two period columns interleaved in the matmul free dim so DMAs stay
  contiguous in the NCHW output layout.
- BitNet quantization is exact in bf16: activations are integers in [-127,127]
  and weights are ternary {-1,0,+1}; both exact in bf16, PSUM accumulates fp32,
  so the tensor engine computes the integer convolution exactly at full bf16
  throughput. Dequantization (s_a*s_w/127 per channel + bias) is one fused
  ScalarE pass reading PSUM; SnakeBeta is Sin+Square ScalarE passes plus one
  VectorE add that also accumulates the next layer's abs-max statistic.
- The per-tensor activation-quant scale needs a global max over the FULL batch:
  each core reduces locally, then a [128]-float AllReduce(max) across the 8
  cores per layer boundary (5 total).
- Layer-1 input quant + all weight quantization replicate the jax reference
  bit-for-bit on host (jax-cpu) inside kernel().
"""

import sys

sys.path.insert(0, "/opt/trn_rl_repo")

import numpy as np
import ml_dtypes

N_CORES = 8
B = 32
BPC = B // N_CORES
MAGIC = 12582912.0  # 1.5 * 2**23: fp32 round-to-nearest-even trick
TT = 256  # output time-steps per psum tile (free dim = 2*TT = 512)
L1_SEG = 1366  # layer-1 im2col segment (output steps)
WQ = 1024  # quant window (time steps)

# (Cin, Cout, K, stride, pad_pairs)
LAYERS = [
    (1, 32, 5, 3, 2),
    (32, 128, 5, 3, 2),
    (128, 512, 5, 3, 2),
    (512, 1024, 5, 3, 2),
    (1024, 1024, 5, 1, 2),
    (1024, 1, 3, 1, 1),
]
GS = [1, 1, 4, 8, 8, 1]  # cout groups of 128


def _tchain(t1):
    ts, t = [], t1
    for (_, _, k, s, p) in LAYERS:
        t = (t + 2 * p - k) // s + 1
        ts.append(t)
    return ts


def _ceil_div(a, b):
    return -(-a // b)


def _build(nc, t1):
    import concourse.mybir as mybir
    import concourse.tile as tile
    import concourse.bass_isa as bass_isa

    dt = mybir.dt
    f32, bf16 = dt.float32, dt.bfloat16
    AX = mybir.AxisListType
    ALU = mybir.AluOpType
    ACT = mybir.ActivationFunctionType

    touts = _tchain(t1)
    tins = [t1] + touts[:-1]

    # ---- DRAM I/O ----
    qx = nc.dram_tensor("qx", [BPC, t1 + 4, 2], bf16, kind="ExternalInput")
    wshapes = [
        [5, 32],
        [32, 5, 128],
        [128, 5, 512],
        [128, 4, 5, 1024],
        [128, 8, 5, 1024],
        [128, 8, 3, 1],
    ]
    wts = [
        nc.dram_tensor(f"w{i + 1}", sh, bf16, kind="ExternalInput")
        for i, sh in enumerate(wshapes)
    ]
    cvec = {}
    for i in range(6):
        g = GS[i]
        for nm in (["bias", "alpha", "sqib"] if i < 5 else ["bias"]):
            cvec[(nm, i)] = nc.dram_tensor(
                f"{nm}{i + 1}", [128, g], f32, kind="ExternalInput"
            )
    cvec[("ds", 0)] = nc.dram_tensor("ds1", [128, 1], f32, kind="ExternalInput")
    for i in range(1, 6):
        cvec[("sw", i)] = nc.dram_tensor(
            f"sw{i + 1}", [128, GS[i]], f32, kind="ExternalInput"
        )

    hs = [
        nc.dram_tensor(
            f"h{i + 1}", [BPC, LAYERS[i][1], touts[i], 2], f32, kind="ExternalOutput"
        )
        for i in range(6)
    ]

    ccs = []
    for i in range(5):
        ci = nc.dram_tensor(f"cc{i}_in", [128], f32, kind="Internal")
        co = nc.dram_tensor(
            f"cc{i}_out", [128], f32, kind="Internal", addr_space="Shared"
        )
        ccs.append((ci, co))

    with tile.TileContext(nc) as tc:
        import contextlib

        with contextlib.ExitStack() as ctx:
            consts = ctx.enter_context(tc.tile_pool(name="consts", bufs=1))
            wsmall = ctx.enter_context(tc.tile_pool(name="wsmall", bufs=1))
            wbig = ctx.enter_context(tc.tile_pool(name="wbig", bufs=2))
            qbpool = ctx.enter_context(tc.tile_pool(name="qbpool", bufs=2))
            zinp = ctx.enter_context(tc.tile_pool(name="zin", bufs=2))
            workp = ctx.enter_context(tc.tile_pool(name="work", bufs=2))
            zoutp = ctx.enter_context(tc.tile_pool(name="zout", bufs=3))
            statp = ctx.enter_context(tc.tile_pool(name="stat", bufs=1))
            psum = ctx.enter_context(tc.tile_pool(name="psum", bufs=6, space="PSUM"))

            # max q-buffer bytes/partition across layers (shared tag -> one slot size)
            l2half = _ceil_div(touts[1], 2)
            QL = max(
                3 * l2half + 2,  # L2 half window (on 128 and 32 partitions)
                max(_ceil_div(LAYERS[i][0], 128) * (tins[i] + 2 * LAYERS[i][4])
                    for i in range(2, 6)),
            )

            cv = {}
            for key, hdl in cvec.items():
                t = consts.tile(list(hdl.shape), f32, tag=f"cv_{key[0]}{key[1]}", name=f"cv_{key[0]}{key[1]}")
                nc.sync.dma_start(t[:], hdl.ap())
                cv[key] = t

            w1s = wsmall.tile([5, 32], bf16, tag="w1")
            nc.sync.dma_start(w1s[:], wts[0].ap())
            w2s = wsmall.tile([32, 5, 128], bf16, tag="w2")
            nc.sync.dma_start(w2s[:], wts[1].ap())
            w3s = wsmall.tile([128, 5, 512], bf16, tag="w3")
            nc.sync.dma_start(w3s[:], wts[2].ap())
            wps = wsmall.tile([128, 8, 3, 1], bf16, tag="wp")
            nc.sync.dma_start(wps[:], wts[5].ap())

            state = {("ds", 0): cv[("ds", 0)]}

            def post_process(i, ps, pg, g, b, t0, tcnt, zmax, slot):
                ds = state[("ds", i)]
                ysb = workp.tile([128, TT, 2], f32, tag="ysb", name="ysb")[:pg, :tcnt, :]
                nc.scalar.activation(
                    ysb, ps, ACT.Identity,
                    bias=cv[("bias", i)][:pg, g : g + 1],
                    scale=ds[:pg, g : g + 1],
                )
                if i < 5:
                    tsb = workp.tile([128, TT, 2], f32, tag="tsb", name="tsb")[:pg, :tcnt, :]
                    nc.scalar.activation(
                        tsb, ysb, ACT.Sin, scale=cv[("alpha", i)][:pg, g : g + 1]
                    )
                    vsb = workp.tile([128, TT, 2], f32, tag="vsb", name="vsb")[:pg, :tcnt, :]
                    nc.scalar.activation(
                        vsb, tsb, ACT.Square, scale=cv[("sqib", i)][:pg, g : g + 1]
                    )
                    zsb = zoutp.tile([128, TT, 2], f32, tag="zsb", name="zsb")[:pg, :tcnt, :]
                    nc.vector.tensor_tensor(zsb, ysb, vsb, ALU.add)
                    nc.vector.tensor_reduce(
                        zmax[:pg, slot : slot + 1], zsb, axis=AX.XY, op=ALU.max,
                        apply_absolute_value=True,
                    )
                else:
                    zsb = ysb
                nc.sync.dma_start(
                    hs[i].ap()[b, g * 128 : g * 128 + pg, t0 : t0 + tcnt, :], zsb
                )

            def finish_layer(i, zmax, nslots):
                zred = statp.tile([128, 1], f32, tag=f"zred{i}")
                nc.vector.tensor_reduce(
                    zred[:], zmax[:, :nslots], axis=AX.X, op=ALU.max
                )
                zall = statp.tile([128, 1], f32, tag=f"zall{i}")
                nc.gpsimd.partition_all_reduce(
                    zall[:], zred[:], 128, bass_isa.ReduceOp.max
                )
                ci, co = ccs[i]
                nc.sync.dma_start(ci.ap(), zall[:, 0])
                nc.gpsimd.collective_compute(
                    "AllReduce", ALU.max,
                    replica_groups=[list(range(N_CORES))],
                    ins=[ci.ap()], outs=[co.ap()],
                )
                sraw = statp.tile([128, 1], f32, tag=f"sraw{i}")
                nc.sync.dma_start(sraw[:], co.ap())
                sb = statp.tile([128, 1], f32, tag=f"sb{i}")
                nc.vector.tensor_scalar_max(sb[:], sraw[:], 1e-5)
                # exactly-rounded fl(127/s) via 2Prod + one Markstein step
                # (reference computes 127.0/s; a double-rounded 127*recip(s)
                # flips ~25% of quant multipliers by 1 ulp -> round() flips)
                def _tt(o, a, b, op):
                    nc.vector.tensor_tensor(o[:], a[:], b[:], op)

                def _tl(nm):
                    return statp.tile([128, 1], f32, tag=f"{nm}{i}", name=nm)

                r0 = _tl("r0")
                nc.vector.reciprocal(r0[:], sb[:])
                y0 = _tl("y0")
                nc.vector.tensor_scalar_mul(y0[:], r0[:], 127.0)
                VC = 4097.0  # Veltkamp split constant 2**12+1
                t1 = _tl("t1"); nc.vector.tensor_scalar_mul(t1[:], sb[:], VC)
                t2 = _tl("t2"); _tt(t2, t1, sb, ALU.subtract)
                ahi = _tl("ahi"); _tt(ahi, t1, t2, ALU.subtract)
                alo = _tl("alo"); _tt(alo, sb, ahi, ALU.subtract)
                u1 = _tl("u1"); nc.vector.tensor_scalar_mul(u1[:], y0[:], VC)
                u2 = _tl("u2"); _tt(u2, u1, y0, ALU.subtract)
                bhi = _tl("bhi"); _tt(bhi, u1, u2, ALU.subtract)
                blo = _tl("blo"); _tt(blo, y0, bhi, ALU.subtract)
                pp_ = _tl("pp_"); _tt(pp_, sb, y0, ALU.mult)
                e = _tl("e")
                _tt(e, ahi, bhi, ALU.mult)
                _tt(e, e, pp_, ALU.subtract)
                t3 = _tl("t3"); _tt(t3, ahi, blo, ALU.mult); _tt(e, e, t3, ALU.add)
                t4 = _tl("t4"); _tt(t4, alo, bhi, ALU.mult); _tt(e, e, t4, ALU.add)
                t5 = _tl("t5"); _tt(t5, alo, blo, ALU.mult); _tt(e, e, t5, ALU.add)
                rho = _tl("rho")
                nc.vector.tensor_scalar(rho[:], pp_[:], -127.0, -1.0, ALU.add, ALU.mult)
                _tt(rho, rho, e, ALU.subtract)
                cor = _tl("cor"); _tt(cor, rho, r0, ALU.mult)
                inv = statp.tile([128, 1], f32, tag=f"inv{i}")
                _tt(inv, y0, cor, ALU.add)
                state[("inv", i + 1)] = inv
                ds = statp.tile([128, GS[i + 1]], f32, tag=f"ds{i + 1}")
                nc.vector.tensor_scalar_mul(ds[:], cv[("sw", i + 1)][:], sb[:, 0:1])
                state[("ds", i + 1)] = ds

            def quantize(i, zin_ap, qdst_ap):
                inv = state[("inv", i)]
                pg = zin_ap.shape[0]
                nc.vector.tensor_scalar(
                    zin_ap, zin_ap, inv[:pg, 0:1], MAGIC, ALU.mult, ALU.add
                )
                nc.vector.tensor_scalar_add(qdst_ap, zin_ap, -MAGIC)

            # ================= layer 1 (im2col, contraction=5) =================
            zmax = statp.tile([128, 256], f32, tag="zmax")
            nc.gpsimd.memset(zmax[:], 0.0)
            slot = 0
            to1 = touts[0]
            with tc.tile_pool(name="imcol", bufs=2) as imcolp:
                for b in range(BPC):
                    for sg in range(_ceil_div(to1, L1_SEG)):
                        s0 = sg * L1_SEG
                        scnt = min(L1_SEG, to1 - s0)
                        wlen = 3 * (scnt - 1) + 1
                        imc = imcolp.tile([5, 3 * L1_SEG, 2], bf16, tag="imcol", name="imcol")
                        for k in range(5):
                            nc.sync.dma_start(
                                imc[k : k + 1, :wlen, :],
                                qx.ap()[b, 3 * s0 + k : 3 * s0 + k + wlen, :],
                            )
                        for tt0 in range(0, scnt, TT):
                            tcnt = min(TT, scnt - tt0)
                            ps = psum.tile([128, TT, 2], f32, tag="ps", name="ps")[:32, :tcnt, :]
                            rhs = imc[:, 3 * tt0 : 3 * (tt0 + tcnt - 1) + 1 : 3, :]
                            nc.tensor.matmul(ps, w1s[:], rhs, start=True, stop=True)
                            post_process(0, ps, 32, 0, b, s0 + tt0, tcnt, zmax, slot)
                            slot += 1
            finish_layer(0, zmax, slot)

            # ================= layer 2 (Cin=32), in two time-halves =============
            zmax = statp.tile([128, 256], f32, tag="zmax")
            nc.gpsimd.memset(zmax[:], 0.0)
            slot = 0
            ti, to = tins[1], touts[1]
            for hh in range(2):
                h0 = hh * l2half
                h1 = min(to, h0 + l2half)
                tlo = 3 * h0 - 2            # logical input t of buffer col 0
                L = 3 * (h1 - 1) + 2 + 1 - tlo
                q1 = qbpool.tile([128, QL, 2], bf16, tag="qsb", name="q1h")[:, :L, :]
                lo_cl = max(tlo, 0)
                hi_cl = min(tlo + L, ti)
                if lo_cl > tlo:
                    nc.gpsimd.memset(q1[:, : lo_cl - tlo, :], 0.0)
                if hi_cl < tlo + L:
                    nc.gpsimd.memset(q1[:, hi_cl - tlo :, :], 0.0)
                for t0 in range(lo_cl, hi_cl, WQ):
                    w = min(WQ, hi_cl - t0)
                    zin = zinp.tile([128, WQ, 2], f32, tag="zin", name="zin")[:, :w, :]
                    nc.sync.dma_start(zin, hs[0].ap()[:, :, t0 : t0 + w, :])
                    quantize(1, zin, q1[:, t0 - tlo : t0 - tlo + w, :])
                for b in range(BPC):
                    q1b = qbpool.tile([32, QL, 2], bf16, tag="qsb", name="q1b")[:, :L, :]
                    nc.sync.dma_start(q1b, q1[32 * b : 32 * b + 32, :, :])
                    for tt0 in range(h0, h1, TT):
                        tcnt = min(TT, h1 - tt0)
                        ps = psum.tile([128, TT, 2], f32, tag="ps", name="ps")[:, :tcnt, :]
                        for k in range(5):
                            a = 3 * (tt0 - h0) + k  # 3*tt0+k-2 - tlo
                            rhs = q1b[:, a : a + 3 * (tcnt - 1) + 1 : 3, :]
                            nc.tensor.matmul(
                                ps, w2s[:, k, :], rhs, start=(k == 0), stop=(k == 4)
                            )
                        post_process(1, ps, 128, 0, b, tt0, tcnt, zmax, slot)
                        slot += 1
            finish_layer(1, zmax, slot)

            # ============ layers 3..6: Cin = 128*chunks ============
            for li in range(2, 6):
                cin, cout, K, stride, pp = LAYERS[li]
                chunks = _ceil_div(cin, 128)
                G = GS[li]
                ti, to = tins[li], touts[li]
                has_snake = li < 5
                if has_snake:
                    zmax = statp.tile([128, 256], f32, tag="zmax")
                    nc.gpsimd.memset(zmax[:], 0.0)
                slot = 0
                if li == 2:
                    pass  # w3s resident
                elif li == 3:
                    w4s = wbig.tile([128, 4, 5, 1024], bf16, tag="wbig")
                    nc.sync.dma_start(w4s[:], wts[3].ap())
                elif li == 4:
                    w5a = wbig.tile([128, 4, 5, 1024], bf16, tag="wbig")
                    nc.sync.dma_start(w5a[:], wts[4].ap()[:, 0:4])
                    w5b = wbig.tile([128, 4, 5, 1024], bf16, tag="wbig")
                    nc.sync.dma_start(w5b[:], wts[4].ap()[:, 4:8])
                for b in range(BPC):
                    qsb = qbpool.tile([128, QL, 2], bf16, tag="qsb", name="qsb")
                    qsb = qsb[:, : chunks * (ti + 2 * pp), :].rearrange(
                        "p (c t) two -> p c t two", c=chunks
                    )
                    nc.gpsimd.memset(qsb[:, :, 0:pp, :], 0.0)
                    nc.gpsimd.memset(qsb[:, :, ti + pp : ti + 2 * pp, :], 0.0)
                    for c in range(chunks):
                        for t0 in range(0, ti, WQ):
                            w = min(WQ, ti - t0)
                            zin = zinp.tile([128, WQ, 2], f32, tag="zin", name="zin")[:, :w, :]
                            nc.sync.dma_start(
                                zin,
                                hs[li - 1].ap()[
                                    b, c * 128 : c * 128 + 128, t0 : t0 + w, :
                                ],
                            )
                            quantize(li, zin, qsb[:, c, pp + t0 : pp + t0 + w, :])
                    for tt0 in range(0, to, TT):
                        tcnt = min(TT, to - tt0)
                        for g in range(G):
                            pg = min(128, cout - g * 128)
                            ps = psum.tile([128, TT, 2], f32, tag="ps", name="ps")[:pg, :tcnt, :]
                            nmm = chunks * K
                            mi = 0
                            for c in range(chunks):
                                for k in range(K):
                                    if li == 2:
                                        lwk = w3s[:, k, g * 128 : g * 128 + pg]
                                    elif li == 3:
                                        lwk = w4s[:, c, k, g * 128 : g * 128 + pg]
                                    elif li == 4:
                                        lwk = (w5a if c < 4 else w5b)[
                                            :, c % 4, k, g * 128 : g * 128 + pg
                                        ]
                                    else:
                                        lwk = wps[:, c, k, :]
                                    a = stride * tt0 + k
                                    rhs = qsb[
                                        :, c, a : a + stride * (tcnt - 1) + 1 : stride, :
                                    ]
                                    nc.tensor.matmul(
                                        ps, lwk, rhs,
                                        start=(mi == 0), stop=(mi == nmm - 1),
                                    )
                                    mi += 1
                            post_process(
                                li, ps, pg, g, b, tt0, tcnt,
                                zmax if has_snake else None, slot,
                            )
                            slot += 1
                if has_snake:
                    finish_layer(li, zmax, slot)
    return touts


def _host_prep(inputs, t1):
    """Replicate reference quantization bit-for-bit on jax-cpu."""
    import jax
    import jax.numpy as jnp

    cpu = jax.devices("cpu")[0]
    x = np.asarray(inputs["x"])
    ws = [inputs[f"w{i + 1}"] for i in range(5)] + [inputs["wp"]]
    bs = [inputs[f"b{i + 1}"] for i in range(5)] + [inputs["bp"]]
    las = [inputs[f"a{i + 1}"] for i in range(5)]
    lbs = [inputs[f"be{i + 1}"] for i in range(5)]

    with jax.default_device(cpu):
        xj = jnp.asarray(x.reshape(B, t1, 2))
        s1 = jnp.maximum(jnp.max(jnp.abs(xj)), 1e-5)
        q = jnp.clip(jnp.round(xj * (127.0 / s1)), -128.0, 127.0)
        qx = np.asarray(q, np.float32)
        s1 = float(np.asarray(s1, np.float32))
        terns, sws = [], []
        for w in ws:
            wj = jnp.asarray(np.asarray(w))
            sw = jnp.maximum(
                jnp.mean(jnp.abs(wj), axis=(1, 2, 3), keepdims=True), 1e-5
            )
            tern = jnp.clip(jnp.round(wj / sw), -1.0, 1.0)
            terns.append(np.asarray(tern, np.float32))
            sws.append(np.asarray(sw, np.float32).reshape(-1))

    qx_pad = np.zeros((B, t1 + 4, 2), np.float32)
    qx_pad[:, 2 : t1 + 2, :] = qx
    qx_pad = qx_pad.astype(ml_dtypes.bfloat16)

    def pack_vec(v, g):
        v = np.asarray(v, np.float32).reshape(-1)
        o = np.zeros((128, g), np.float32)
        o[np.arange(v.size) % 128, np.arange(v.size) // 128] = v
        return o

    feed = {}
    feed["w1"] = terns[0][:, 0, :, 0].T.astype(ml_dtypes.bfloat16)
    feed["w2"] = terns[1][:, :, :, 0].transpose(1, 2, 0).astype(ml_dtypes.bfloat16)
    feed["w3"] = terns[2][:, :, :, 0].transpose(1, 2, 0).astype(ml_dtypes.bfloat16)
    feed["w4"] = np.ascontiguousarray(
        terns[3][:, :, :, 0].transpose(1, 2, 0).reshape(4, 128, 5, 1024)
        .transpose(1, 0, 2, 3)
    ).astype(ml_dtypes.bfloat16)
    feed["w5"] = np.ascontiguousarray(
        terns[4][:, :, :, 0].transpose(1, 2, 0).reshape(8, 128, 5, 1024)
        .transpose(1, 0, 2, 3)
    ).astype(ml_dtypes.bfloat16)
    feed["w6"] = np.ascontiguousarray(
        terns[5][:, :, :, 0].transpose(1, 2, 0).reshape(8, 128, 3, 1)
        .transpose(1, 0, 2, 3)
    ).astype(ml_dtypes.bfloat16)

    for i in range(6):
        feed[f"bias{i + 1}"] = pack_vec(bs[i], GS[i])
        if i < 5:
            feed[f"alpha{i + 1}"] = pack_vec(
                np.exp(np.asarray(las[i], np.float32)), GS[i]
            )
            ib = 1.0 / (np.exp(np.asarray(lbs[i], np.float32)) + 1e-9)
            feed[f"sqib{i + 1}"] = pack_vec(np.sqrt(ib).astype(np.float32), GS[i])
    feed["ds1"] = pack_vec((s1 / 127.0) * sws[0], 1)
    for i in range(1, 6):
        feed[f"sw{i + 1}"] = pack_vec(sws[i] / 127.0, GS[i])
    return feed, qx_pad


_CACHE = {}


def _get_compiled(t1):
    if t1 in _CACHE:
        return _CACHE[t1]
    from concourse import bacc
    from concourse.bass_interp import get_hw_module

    nc = bacc.Bacc(
        "TRN2", target_bir_lowering=False, debug=False,
        enable_asserts=True, num_devices=N_CORES,
    )
    _build(nc, t1)
    nc.compile()
    nc.m = get_hw_module(nc.m)
    _CACHE[t1] = nc
    return nc


def kernel(**inputs):
    from concourse import bass_utils

    t1 = inputs["x"].shape[2] // 2
    feed, qx_pad = _host_prep(inputs, t1)
    nc = _get_compiled(t1)

    in_maps = []
    for c in range(N_CORES):
        m = dict(feed)
        m["qx"] = np.ascontiguousarray(qx_pad[c * BPC : (c + 1) * BPC])
        in_maps.append(m)

    res = bass_utils.run_bass_kernel_spmd(nc, in_maps, core_ids=list(range(N_CORES)))
    outs = res.results
    hs = [
        np.concatenate([outs[c][f"h{i + 1}"] for c in range(N_CORES)], axis=0)
        for i in range(6)
    ]
    out = hs[5].reshape(B, -1)
    return (out, tuple(hs))


# revision 11
# speedup vs baseline: 1.2945x; 1.2945x over previous
"""Trainium2 Bass kernel for nn_BitDiscriminatorP (BitNet conv discriminator, period 2).

Strategy:
- Pure data parallelism: batch 32 sharded 4-per-core across 8 NeuronCores.
- All convs are 1-D convs over H (the period dim has kernel 1) -> expressed as
  accumulating matmuls on the tensor engine with stride-3 access patterns, with
  the two period columns interleaved in the matmul free dim so DMAs stay
  contiguous in the NCHW output layout.
- BitNet quantization is exact in bf16: activations are integers in [-127,127]
  and weights are ternary {-1,0,+1}; both exact in bf16, PSUM accumulates fp32,
  so the tensor engine computes the integer convolution exactly at full bf16
  throughput. Dequantization (s_a*s_w/127 per channel + bias) is one fused
  ScalarE pass reading PSUM; SnakeBeta is Sin+Square ScalarE passes plus one
  VectorE add that also accumulates the next layer's abs-max statistic.
- The per-tensor activation-quant scale needs a global max over the FULL batch:
  each core reduces locally, then a [128]-float AllReduce(max) across the 8
  cores per layer boundary (5 total).
- Layer-1 input quant + all weight quantization replicate the jax reference
  bit-for-bit on host (jax-cpu) inside kernel().
"""

import sys

sys.path.insert(0, "/opt/trn_rl_repo")

import numpy as np
import ml_dtypes

N_CORES = 8
B = 32
BPC = B // N_CORES
MAGIC = 12582912.0  # 1.5 * 2**23: fp32 round-to-nearest-even trick
TT = 256  # output time-steps per psum tile (free dim = 2*TT = 512)
L1_SEG = 512  # layer-1 im2col segment (output steps)
WQ = 1024  # quant window (time steps)

# (Cin, Cout, K, stride, pad_pairs)
LAYERS = [
    (1, 32, 5, 3, 2),
    (32, 128, 5, 3, 2),
    (128, 512, 5, 3, 2),
    (512, 1024, 5, 3, 2),
    (1024, 1024, 5, 1, 2),
    (1024, 1, 3, 1, 1),
]
GS = [1, 1, 4, 8, 8, 1]  # cout groups of 128


def _tchain(t1):
    ts, t = [], t1
    for (_, _, k, s, p) in LAYERS:
        t = (t + 2 * p - k) // s + 1
        ts.append(t)
    return ts


def _ceil_div(a, b):
    return -(-a // b)


def _build(nc, t1, no_collective=False):
    import concourse.mybir as mybir
    import concourse.tile as tile
    import concourse.bass_isa as bass_isa

    dt = mybir.dt
    f32, bf16 = dt.float32, dt.bfloat16
    AX = mybir.AxisListType
    ALU = mybir.AluOpType
    ACT = mybir.ActivationFunctionType

    touts = _tchain(t1)
    tins = [t1] + touts[:-1]

    # ---- DRAM I/O ----
    qx = nc.dram_tensor("qx", [BPC, t1 + 4, 2], bf16, kind="ExternalInput")
    wshapes = [
        [5, 32],
        [32, 5, 128],
        [128, 5, 512],
        [128, 4, 5, 1024],
        [128, 8, 5, 1024],
        [128, 8, 3, 1],
    ]
    wts = [
        nc.dram_tensor(f"w{i + 1}", sh, bf16, kind="ExternalInput")
        for i, sh in enumerate(wshapes)
    ]
    cvec = {}
    for i in range(6):
        g = GS[i]
        for nm in (["bias", "alpha", "sqib"] if i < 5 else ["bias"]):
            cvec[(nm, i)] = nc.dram_tensor(
                f"{nm}{i + 1}", [128, g], f32, kind="ExternalInput"
            )
    cvec[("ds", 0)] = nc.dram_tensor("ds1", [128, 1], f32, kind="ExternalInput")
    for i in range(1, 6):
        cvec[("sw", i)] = nc.dram_tensor(
            f"sw{i + 1}", [128, GS[i]], f32, kind="ExternalInput"
        )

    hs = [
        nc.dram_tensor(
            f"h{i + 1}", [BPC, LAYERS[i][1], touts[i], 2], f32, kind="ExternalOutput"
        )
        for i in range(6)
    ]

    ccs = []
    for i in range(5):
        ci = nc.dram_tensor(f"cc{i}_in", [128], f32, kind="Internal")
        co = nc.dram_tensor(
            f"cc{i}_out", [128], f32, kind="Internal", addr_space="Shared"
        )
        ccs.append((ci, co))

    with tile.TileContext(nc) as tc:
        import contextlib

        with contextlib.ExitStack() as ctx:
            consts = ctx.enter_context(tc.tile_pool(name="consts", bufs=1))
            wsmall = ctx.enter_context(tc.tile_pool(name="wsmall", bufs=1))
            wbig = ctx.enter_context(tc.tile_pool(name="wbig", bufs=2))
            qbpool = ctx.enter_context(tc.tile_pool(name="qbpool", bufs=2))
            zinp = ctx.enter_context(tc.tile_pool(name="zin", bufs=2))
            workp = ctx.enter_context(tc.tile_pool(name="work", bufs=2))
            zoutp = ctx.enter_context(tc.tile_pool(name="zout", bufs=3))
            statp = ctx.enter_context(tc.tile_pool(name="stat", bufs=1))
            psum = ctx.enter_context(tc.tile_pool(name="psum", bufs=6, space="PSUM"))

            # max q-buffer bytes/partition across layers (shared tag -> one slot size)
            l2half = _ceil_div(touts[1], 2)
            QL = max(
                3 * l2half + 2,  # L2 half window (on 128 and 32 partitions)
                max(_ceil_div(LAYERS[i][0], 128) * (tins[i] + 2 * LAYERS[i][4])
                    for i in range(2, 6)),
            )

            cv = {}
            for key, hdl in cvec.items():
                t = consts.tile(list(hdl.shape), f32, tag=f"cv_{key[0]}{key[1]}", name=f"cv_{key[0]}{key[1]}")
                nc.sync.dma_start(t[:], hdl.ap())
                cv[key] = t

            w1s = wsmall.tile([5, 32], bf16, tag="w1")
            nc.sync.dma_start(w1s[:], wts[0].ap())
            w2s = wsmall.tile([32, 5, 128], bf16, tag="w2")
            nc.sync.dma_start(w2s[:], wts[1].ap())
            w3s = wsmall.tile([128, 5, 512], bf16, tag="w3")
            nc.sync.dma_start(w3s[:], wts[2].ap())
            wps = wsmall.tile([128, 8, 3, 1], bf16, tag="wp")
            nc.sync.dma_start(wps[:], wts[5].ap())

            state = {("ds", 0): cv[("ds", 0)]}

            def post_process(i, ps, pg, g, b, t0, tcnt, zmax, slot):
                ds = state[("ds", i)]
                ysb = workp.tile([128, TT, 2], f32, tag="ysb", name="ysb")[:pg, :tcnt, :]
                nc.scalar.activation(
                    ysb, ps, ACT.Identity,
                    bias=cv[("bias", i)][:pg, g : g + 1],
                    scale=ds[:pg, g : g + 1],
                )
                if i < 5:
                    tsb = workp.tile([128, TT, 2], f32, tag="tsb", name="tsb")[:pg, :tcnt, :]
                    nc.scalar.activation(
                        tsb, ysb, ACT.Sin, scale=cv[("alpha", i)][:pg, g : g + 1]
                    )
                    vsb = workp.tile([128, TT, 2], f32, tag="vsb", name="vsb")[:pg, :tcnt, :]
                    nc.scalar.activation(
                        vsb, tsb, ACT.Square, scale=cv[("sqib", i)][:pg, g : g + 1]
                    )
                    zsb = zoutp.tile([128, TT, 2], f32, tag="zsb", name="zsb")[:pg, :tcnt, :]
                    nc.vector.tensor_tensor(zsb, ysb, vsb, ALU.add)
                    nc.vector.tensor_reduce(
                        zmax[:pg, slot : slot + 1], zsb, axis=AX.XY, op=ALU.max,
                        apply_absolute_value=True,
                    )
                else:
                    zsb = ysb
                if b is None:  # layer-1: partitions are (b, cout) packed
                    dst = hs[i].ap()[:, :, t0 : t0 + tcnt, :]
                else:
                    dst = hs[i].ap()[b, g * 128 : g * 128 + pg, t0 : t0 + tcnt, :]
                nc.sync.dma_start(dst, zsb)

            def finish_layer(i, zmax, nslots):
                zred = statp.tile([128, 1], f32, tag=f"zred{i}")
                nc.vector.tensor_reduce(
                    zred[:], zmax[:, :nslots], axis=AX.X, op=ALU.max
                )
                zall = statp.tile([128, 1], f32, tag=f"zall{i}")
                nc.gpsimd.partition_all_reduce(
                    zall[:], zred[:], 128, bass_isa.ReduceOp.max
                )
                ci, co = ccs[i]
                nc.sync.dma_start(ci.ap(), zall[:, 0])
                if not no_collective:
                    nc.gpsimd.collective_compute(
                        "AllReduce", ALU.max,
                        replica_groups=[list(range(N_CORES))],
                        ins=[ci.ap()], outs=[co.ap()],
                    )
                sraw = statp.tile([128, 1], f32, tag=f"sraw{i}")
                nc.sync.dma_start(sraw[:], (ci if no_collective else co).ap())
                sb = statp.tile([128, 1], f32, tag=f"sb{i}")
                nc.vector.tensor_scalar_max(sb[:], sraw[:], 1e-5)
                # exactly-rounded fl(127/s) via 2Prod + one Markstein step
                # (reference computes 127.0/s; a double-rounded 127*recip(s)
                # flips ~25% of quant multipliers by 1 ulp -> round() flips)
                def _tt(o, a, b, op):
                    nc.vector.tensor_tensor(o[:], a[:], b[:], op)

                def _tl(nm):
                    return statp.tile([128, 1], f32, tag=f"{nm}{i}", name=nm)

                r0 = _tl("r0")
                nc.vector.reciprocal(r0[:], sb[:])
                y0 = _tl("y0")
                nc.vector.tensor_scalar_mul(y0[:], r0[:], 127.0)
                VC = 4097.0  # Veltkamp split constant 2**12+1
                t1 = _tl("t1"); nc.vector.tensor_scalar_mul(t1[:], sb[:], VC)
                t2 = _tl("t2"); _tt(t2, t1, sb, ALU.subtract)
                ahi = _tl("ahi"); _tt(ahi, t1, t2, ALU.subtract)
                alo = _tl("alo"); _tt(alo, sb, ahi, ALU.subtract)
                u1 = _tl("u1"); nc.vector.tensor_scalar_mul(u1[:], y0[:], VC)
                u2 = _tl("u2"); _tt(u2, u1, y0, ALU.subtract)
                bhi = _tl("bhi"); _tt(bhi, u1, u2, ALU.subtract)
                blo = _tl("blo"); _tt(blo, y0, bhi, ALU.subtract)
                pp_ = _tl("pp_"); _tt(pp_, sb, y0, ALU.mult)
                e = _tl("e")
                _tt(e, ahi, bhi, ALU.mult)
                _tt(e, e, pp_, ALU.subtract)
                t3 = _tl("t3"); _tt(t3, ahi, blo, ALU.mult); _tt(e, e, t3, ALU.add)
                t4 = _tl("t4"); _tt(t4, alo, bhi, ALU.mult); _tt(e, e, t4, ALU.add)
                t5 = _tl("t5"); _tt(t5, alo, blo, ALU.mult); _tt(e, e, t5, ALU.add)
                rho = _tl("rho")
                nc.vector.tensor_scalar(rho[:], pp_[:], -127.0, -1.0, ALU.add, ALU.mult)
                _tt(rho, rho, e, ALU.subtract)
                cor = _tl("cor"); _tt(cor, rho, r0, ALU.mult)
                inv = statp.tile([128, 1], f32, tag=f"inv{i}")
                _tt(inv, y0, cor, ALU.add)
                state[("inv", i + 1)] = inv
                ds = statp.tile([128, GS[i + 1]], f32, tag=f"ds{i + 1}")
                nc.vector.tensor_scalar_mul(ds[:], cv[("sw", i + 1)][:], sb[:, 0:1])
                state[("ds", i + 1)] = ds

            def quantize(i, zin_ap, qdst_ap):
                inv = state[("inv", i)]
                pg = zin_ap.shape[0]
                nc.vector.tensor_scalar(
                    zin_ap, zin_ap, inv[:pg, 0:1], MAGIC, ALU.mult, ALU.add
                )
                nc.vector.tensor_scalar_add(qdst_ap, zin_ap, -MAGIC)

            # ================= layer 1 (im2col, contraction=5) =================
            zmax = statp.tile([128, 256], f32, tag="zmax")
            nc.gpsimd.memset(zmax[:], 0.0)
            slot = 0
            to1 = touts[0]
            with tc.tile_pool(name="imcol", bufs=5) as imcolp:
                for sg in range(_ceil_div(to1, L1_SEG)):
                    s0 = sg * L1_SEG
                    scnt = min(L1_SEG, to1 - s0)
                    wlen = 3 * (scnt - 1) + 1
                    imcs = []
                    for b in range(BPC):
                        imc = imcolp.tile(
                            [5, 3 * L1_SEG, 2], bf16, tag="imcol", name="imcol"
                        )
                        for k in range(5):
                            nc.sync.dma_start(
                                imc[k : k + 1, :wlen, :],
                                qx.ap()[b, 3 * s0 + k : 3 * s0 + k + wlen, :],
                            )
                        imcs.append(imc)
                    for tt0 in range(0, scnt, TT):
                        tcnt = min(TT, scnt - tt0)
                        ps = psum.tile([128, TT, 2], f32, tag="ps", name="ps")[:, :tcnt, :]
                        for b in range(BPC):
                            rhs = imcs[b][:, 3 * tt0 : 3 * (tt0 + tcnt - 1) + 1 : 3, :]
                            nc.tensor.matmul(
                                ps[32 * b : 32 * b + 32], w1s[:], rhs,
                                start=True, stop=True,
                                tile_position=(0, 32 * b),
                                skip_group_check=True,
                            )
                        post_process(0, ps, 128, 0, None, s0 + tt0, tcnt, zmax, slot)
                        slot += 1
            finish_layer(0, zmax, slot)

            # ================= layer 2 (Cin=32), in two time-halves =============
            zmax = statp.tile([128, 256], f32, tag="zmax")
            nc.gpsimd.memset(zmax[:], 0.0)
            slot = 0
            ti, to = tins[1], touts[1]
            for hh in range(2):
                h0 = hh * l2half
                h1 = min(to, h0 + l2half)
                tlo = 3 * h0 - 2            # logical input t of buffer col 0
                L = 3 * (h1 - 1) + 2 + 1 - tlo
                q1 = qbpool.tile([128, QL, 2], bf16, tag="qsb", name="q1h")[:, :L, :]
                lo_cl = max(tlo, 0)
                hi_cl = min(tlo + L, ti)
                if lo_cl > tlo:
                    nc.gpsimd.memset(q1[:, : lo_cl - tlo, :], 0.0)
                if hi_cl < tlo + L:
                    nc.gpsimd.memset(q1[:, hi_cl - tlo :, :], 0.0)
                for t0 in range(lo_cl, hi_cl, WQ):
                    w = min(WQ, hi_cl - t0)
                    zin = zinp.tile([128, WQ, 2], f32, tag="zin", name="zin")[:, :w, :]
                    nc.sync.dma_start(zin, hs[0].ap()[:, :, t0 : t0 + w, :])
                    quantize(1, zin, q1[:, t0 - tlo : t0 - tlo + w, :])
                for b in range(BPC):
                    q1b = qbpool.tile([32, QL, 2], bf16, tag="qsb", name="q1b")[:, :L, :]
                    nc.sync.dma_start(q1b, q1[32 * b : 32 * b + 32, :, :])
                    for tt0 in range(h0, h1, TT):
                        tcnt = min(TT, h1 - tt0)
                        ps = psum.tile([128, TT, 2], f32, tag="ps", name="ps")[:, :tcnt, :]
                        for k in range(5):
                            a = 3 * (tt0 - h0) + k  # 3*tt0+k-2 - tlo
                            rhs = q1b[:, a : a + 3 * (tcnt - 1) + 1 : 3, :]
                            nc.tensor.matmul(
                                ps, w2s[:, k, :], rhs, start=(k == 0), stop=(k == 4)
                            )
                        post_process(1, ps, 128, 0, b, tt0, tcnt, zmax, slot)
                        slot += 1
            finish_layer(1, zmax, slot)

            # ============ layers 3..6: Cin = 128*chunks ============
            for li in range(2, 6):
                cin, cout, K, stride, pp = LAYERS[li]
                chunks = _ceil_div(cin, 128)
                G = GS[li]
                ti, to = tins[li], touts[li]
                has_snake = li < 5
                if has_snake:
                    zmax = statp.tile([128, 256], f32, tag="zmax")
                    nc.gpsimd.memset(zmax[:], 0.0)
                slot = 0
                if li == 2:
                    pass  # w3s resident
                elif li == 3:
                    w4s = wbig.tile([128, 4, 5, 1024], bf16, tag="wbig")
                    nc.sync.dma_start(w4s[:], wts[3].ap())
                elif li == 4:
                    w5a = wbig.tile([128, 4, 5, 1024], bf16, tag="wbig")
                    nc.sync.dma_start(w5a[:], wts[4].ap()[:, 0:4])
                    w5b = wbig.tile([128, 4, 5, 1024], bf16, tag="wbig")
                    nc.sync.dma_start(w5b[:], wts[4].ap()[:, 4:8])
                for b in range(BPC):
                    qsb = qbpool.tile([128, QL, 2], bf16, tag="qsb", name="qsb")
                    qsb = qsb[:, : chunks * (ti + 2 * pp), :].rearrange(
                        "p (c t) two -> p c t two", c=chunks
                    )
                    nc.gpsimd.memset(qsb[:, :, 0:pp, :], 0.0)
                    nc.gpsimd.memset(qsb[:, :, ti + pp : ti + 2 * pp, :], 0.0)
                    for c in range(chunks):
                        for t0 in range(0, ti, WQ):
                            w = min(WQ, ti - t0)
                            zin = zinp.tile([128, WQ, 2], f32, tag="zin", name="zin")[:, :w, :]
                            nc.sync.dma_start(
                                zin,
                                hs[li - 1].ap()[
                                    b, c * 128 : c * 128 + 128, t0 : t0 + w, :
                                ],
                            )
                            quantize(li, zin, qsb[:, c, pp + t0 : pp + t0 + w, :])
                    for tt0 in range(0, to, TT):
                        tcnt = min(TT, to - tt0)
                        for g in range(G):
                            pg = min(128, cout - g * 128)
                            ps = psum.tile([128, TT, 2], f32, tag="ps", name="ps")[:pg, :tcnt, :]
                            nmm = chunks * K
                            mi = 0
                            for c in range(chunks):
                                for k in range(K):
                                    if li == 2:
                                        lwk = w3s[:, k, g * 128 : g * 128 + pg]
                                    elif li == 3:
                                        lwk = w4s[:, c, k, g * 128 : g * 128 + pg]
                                    elif li == 4:
                                        lwk = (w5a if c < 4 else w5b)[
                                            :, c % 4, k, g * 128 : g * 128 + pg
                                        ]
                                    else:
                                        lwk = wps[:, c, k, :]
                                    a = stride * tt0 + k
                                    rhs = qsb[
                                        :, c, a : a + stride * (tcnt - 1) + 1 : stride, :
                                    ]
                                    nc.tensor.matmul(
                                        ps, lwk, rhs,
                                        start=(mi == 0), stop=(mi == nmm - 1),
                                    )
                                    mi += 1
                            post_process(
                                li, ps, pg, g, b, tt0, tcnt,
                                zmax if has_snake else None, slot,
                            )
                            slot += 1
                if has_snake:
                    finish_layer(li, zmax, slot)
    return touts


def _host_prep(inputs, t1):
    """Replicate reference quantization bit-for-bit on jax-cpu."""
    import jax
    import jax.numpy as jnp

    cpu = jax.devices("cpu")[0]
    x = np.asarray(inputs["x"])
    ws = [inputs[f"w{i + 1}"] for i in range(5)] + [inputs["wp"]]
    bs = [inputs[f"b{i + 1}"] for i in range(5)] + [inputs["bp"]]
    las = [inputs[f"a{i + 1}"] for i in range(5)]
    lbs = [inputs[f"be{i + 1}"] for i in range(5)]

    with jax.default_device(cpu):
        xj = jnp.asarray(x.reshape(B, t1, 2))
        s1 = jnp.maximum(jnp.max(jnp.abs(xj)), 1e-5)
        q = jnp.clip(jnp.round(xj * (127.0 / s1)), -128.0, 127.0)
        qx = np.asarray(q, np.float32)
        s1 = float(np.asarray(s1, np.float32))
        terns, sws = [], []
        for w in ws:
            wj = jnp.asarray(np.asarray(w))
            sw = jnp.maximum(
                jnp.mean(jnp.abs(wj), axis=(1, 2, 3), keepdims=True), 1e-5
            )
            tern = jnp.clip(jnp.round(wj / sw), -1.0, 1.0)
            terns.append(np.asarray(tern, np.float32))
            sws.append(np.asarray(sw, np.float32).reshape(-1))

    qx_pad = np.zeros((B, t1 + 4, 2), np.float32)
    qx_pad[:, 2 : t1 + 2, :] = qx
    qx_pad = qx_pad.astype(ml_dtypes.bfloat16)

    def pack_vec(v, g):
        v = np.asarray(v, np.float32).reshape(-1)
        o = np.zeros((128, g), np.float32)
        o[np.arange(v.size) % 128, np.arange(v.size) // 128] = v
        return o

    feed = {}
    feed["w1"] = terns[0][:, 0, :, 0].T.astype(ml_dtypes.bfloat16)
    feed["w2"] = terns[1][:, :, :, 0].transpose(1, 2, 0).astype(ml_dtypes.bfloat16)
    feed["w3"] = terns[2][:, :, :, 0].transpose(1, 2, 0).astype(ml_dtypes.bfloat16)
    feed["w4"] = np.ascontiguousarray(
        terns[3][:, :, :, 0].transpose(1, 2, 0).reshape(4, 128, 5, 1024)
        .transpose(1, 0, 2, 3)
    ).astype(ml_dtypes.bfloat16)
    feed["w5"] = np.ascontiguousarray(
        terns[4][:, :, :, 0].transpose(1, 2, 0).reshape(8, 128, 5, 1024)
        .transpose(1, 0, 2, 3)
    ).astype(ml_dtypes.bfloat16)
    feed["w6"] = np.ascontiguousarray(
        terns[5][:, :, :, 0].transpose(1, 2, 0).reshape(8, 128, 3, 1)
        .transpose(1, 0, 2, 3)
    ).astype(ml_dtypes.bfloat16)

    for i in range(6):
        rep = (lambda v: np.tile(np.asarray(v, np.float32), BPC)) if i == 0 else (lambda v: v)
        feed[f"bias{i + 1}"] = pack_vec(rep(bs[i]), GS[i])
        if i < 5:
            feed[f"alpha{i + 1}"] = pack_vec(
                rep(np.exp(np.asarray(las[i], np.float32))), GS[i]
            )
            ib = 1.0 / (np.exp(np.asarray(lbs[i], np.float32)) + 1e-9)
            feed[f"sqib{i + 1}"] = pack_vec(rep(np.sqrt(ib).astype(np.float32)), GS[i])
    feed["ds1"] = pack_vec(np.tile((s1 / 127.0) * sws[0], BPC), 1)
    for i in range(1, 6):
        feed[f"sw{i + 1}"] = pack_vec(sws[i] / 127.0, GS[i])
    return feed, qx_pad


_CACHE = {}


def _get_compiled(t1):
    if t1 in _CACHE:
        return _CACHE[t1]
    from concourse import bacc
    from concourse.bass_interp import get_hw_module

    nc = bacc.Bacc(
        "TRN2", target_bir_lowering=False, debug=False,
        enable_asserts=True, num_devices=N_CORES,
    )
    _build(nc, t1)
    nc.compile()
    nc.m = get_hw_module(nc.m)
    _CACHE[t1] = nc
    return nc


def kernel(**inputs):
    from concourse import bass_utils

    t1 = inputs["x"].shape[2] // 2
    feed, qx_pad = _host_prep(inputs, t1)
    nc = _get_compiled(t1)

    in_maps = []
    for c in range(N_CORES):
        m = dict(feed)
        m["qx"] = np.ascontiguousarray(qx_pad[c * BPC : (c + 1) * BPC])
        in_maps.append(m)

    res = bass_utils.run_bass_kernel_spmd(nc, in_maps, core_ids=list(range(N_CORES)))
    outs = res.results
    hs = [
        np.concatenate([outs[c][f"h{i + 1}"] for c in range(N_CORES)], axis=0)
        for i in range(6)
    ]
    out = hs[5].reshape(B, -1)
    return (out, tuple(hs))


# revision 17
# speedup vs baseline: 1.4602x; 1.1281x over previous
"""Trainium2 Bass kernel for nn_BitDiscriminatorP (BitNet conv discriminator, period 2).

Strategy:
- Pure data parallelism: batch 32 sharded 4-per-core across 8 NeuronCores.
- All convs are 1-D convs over H (the period dim has kernel 1) -> expressed as
  accumulating matmuls on the tensor engine with stride-3 access patterns, with
  the two period columns interleaved in the matmul free dim so DMAs stay
  contiguous in the NCHW output layout.
- BitNet quantization is exact in bf16: activations are integers in [-127,127]
  and weights are ternary {-1,0,+1}; both exact in bf16, PSUM accumulates fp32,
  so the tensor engine computes the integer convolution exactly at full bf16
  throughput. Dequantization (s_a*s_w/127 per channel + bias) is one fused
  ScalarE pass reading PSUM; SnakeBeta is Sin+Square ScalarE passes plus one
  VectorE add that also accumulates the next layer's abs-max statistic.
- The per-tensor activation-quant scale needs a global max over the FULL batch:
  each core reduces locally, then a [128]-float AllReduce(max) across the 8
  cores per layer boundary (5 total).
- Layer-1 input quant + all weight quantization replicate the jax reference
  bit-for-bit on host (jax-cpu) inside kernel().
"""

import sys

sys.path.insert(0, "/opt/trn_rl_repo")

import numpy as np
import ml_dtypes

N_CORES = 8
B = 32
BPC = B // N_CORES
MAGIC = 12582912.0  # 1.5 * 2**23: fp32 round-to-nearest-even trick
TT = 256  # output time-steps per psum tile (free dim = 2*TT = 512)
L1_SEG = 512  # layer-1 im2col segment (output steps)
WQ = 1024  # quant window (time steps)

# (Cin, Cout, K, stride, pad_pairs)
LAYERS = [
    (1, 32, 5, 3, 2),
    (32, 128, 5, 3, 2),
    (128, 512, 5, 3, 2),
    (512, 1024, 5, 3, 2),
    (1024, 1024, 5, 1, 2),
    (1024, 1, 3, 1, 1),
]
GS = [1, 1, 4, 8, 8, 1]  # cout groups of 128


def _tchain(t1):
    ts, t = [], t1
    for (_, _, k, s, p) in LAYERS:
        t = (t + 2 * p - k) // s + 1
        ts.append(t)
    return ts


def _ceil_div(a, b):
    return -(-a // b)


def _build(nc, t1, no_collective=False):
    import concourse.mybir as mybir
    import concourse.tile as tile
    import concourse.bass_isa as bass_isa

    dt = mybir.dt
    f32, bf16 = dt.float32, dt.bfloat16
    AX = mybir.AxisListType
    ALU = mybir.AluOpType
    ACT = mybir.ActivationFunctionType

    touts = _tchain(t1)
    tins = [t1] + touts[:-1]

    # ---- DRAM I/O ----
    qx = nc.dram_tensor("qx", [BPC, t1 + 4, 2], bf16, kind="ExternalInput")
    wshapes = [
        [5, 32],
        [32, 5, 128],
        [128, 5, 512],
        [128, 4, 5, 1024],
        [128, 8, 5, 1024],
        [128, 8, 3, 1],
    ]
    wts = [
        nc.dram_tensor(f"w{i + 1}", sh, bf16, kind="ExternalInput")
        for i, sh in enumerate(wshapes)
    ]
    cvec = {}
    for i in range(6):
        g = GS[i]
        for nm in (["bias", "alpha", "sqib"] if i < 5 else ["bias"]):
            cvec[(nm, i)] = nc.dram_tensor(
                f"{nm}{i + 1}", [128, g], f32, kind="ExternalInput"
            )
    cvec[("ds", 0)] = nc.dram_tensor("ds1", [128, 1], f32, kind="ExternalInput")
    for i in range(1, 6):
        cvec[("sw", i)] = nc.dram_tensor(
            f"sw{i + 1}", [128, GS[i]], f32, kind="ExternalInput"
        )

    hs = [
        nc.dram_tensor(
            f"h{i + 1}", [BPC, LAYERS[i][1], touts[i], 2], f32, kind="ExternalOutput"
        )
        for i in range(6)
    ]

    ccs = []
    for i in range(5):
        ci = nc.dram_tensor(f"cc{i}_in", [128], f32, kind="Internal")
        co = nc.dram_tensor(
            f"cc{i}_out", [128], f32, kind="Internal", addr_space="Shared"
        )
        ccs.append((ci, co))

    with tile.TileContext(nc) as tc:
        import contextlib

        with contextlib.ExitStack() as ctx:
            consts = ctx.enter_context(tc.tile_pool(name="consts", bufs=1))
            wsmall = ctx.enter_context(tc.tile_pool(name="wsmall", bufs=1))
            wbig = ctx.enter_context(tc.tile_pool(name="wbig", bufs=2))
            qbpool = ctx.enter_context(tc.tile_pool(name="qbpool", bufs=2))
            zinp = ctx.enter_context(tc.tile_pool(name="zin", bufs=3))
            workp = ctx.enter_context(tc.tile_pool(name="work", bufs=2))
            zoutp = ctx.enter_context(tc.tile_pool(name="zout", bufs=3))
            statp = ctx.enter_context(tc.tile_pool(name="stat", bufs=1))
            psum = ctx.enter_context(tc.tile_pool(name="psum", bufs=6, space="PSUM"))

            # max q-buffer bytes/partition across layers (shared tag -> one slot size)
            l2half = _ceil_div(touts[1], 2)
            QL = max(
                3 * l2half + 2,  # L2 half window (on 128 and 32 partitions)
                max(_ceil_div(LAYERS[i][0], 128) * (tins[i] + 2 * LAYERS[i][4])
                    for i in range(2, 6)),
            )

            cv = {}
            for key, hdl in cvec.items():
                t = consts.tile(list(hdl.shape), f32, tag=f"cv_{key[0]}{key[1]}", name=f"cv_{key[0]}{key[1]}")
                nc.sync.dma_start(t[:], hdl.ap())
                cv[key] = t

            w1s = wsmall.tile([5, 32], bf16, tag="w1")
            nc.sync.dma_start(w1s[:], wts[0].ap())
            w2s = wsmall.tile([32, 5, 128], bf16, tag="w2")
            nc.sync.dma_start(w2s[:], wts[1].ap())
            w3s = wsmall.tile([128, 5, 512], bf16, tag="w3")
            nc.sync.dma_start(w3s[:], wts[2].ap())
            wps = wsmall.tile([128, 8, 3, 1], bf16, tag="wp")
            nc.sync.dma_start(wps[:], wts[5].ap())

            state = {("ds", 0): cv[("ds", 0)]}

            def post_process(i, ps, pg, g, b, t0, tcnt, zmax, slot):
                ds = state[("ds", i)]
                ysb = workp.tile([128, TT, 2], f32, tag="ysb", name="ysb")[:pg, :tcnt, :]
                nc.scalar.activation(
                    ysb, ps, ACT.Identity,
                    bias=cv[("bias", i)][:pg, g : g + 1],
                    scale=ds[:pg, g : g + 1],
                )
                if i < 5:
                    tsb = workp.tile([128, TT, 2], f32, tag="tsb", name="tsb")[:pg, :tcnt, :]
                    nc.scalar.activation(
                        tsb, ysb, ACT.Sin, scale=cv[("alpha", i)][:pg, g : g + 1]
                    )
                    vsb = workp.tile([128, TT, 2], f32, tag="vsb", name="vsb")[:pg, :tcnt, :]
                    nc.scalar.activation(
                        vsb, tsb, ACT.Square, scale=cv[("sqib", i)][:pg, g : g + 1]
                    )
                    zsb = zoutp.tile([128, TT, 2], f32, tag="zsb", name="zsb")[:pg, :tcnt, :]
                    nc.vector.tensor_tensor(zsb, ysb, vsb, ALU.add)
                    nc.vector.tensor_reduce(
                        zmax[:pg, slot : slot + 1], zsb, axis=AX.XY, op=ALU.max,
                        apply_absolute_value=True,
                    )
                else:
                    zsb = ysb
                if b is None:  # layer-1: partitions are (b, cout) packed
                    dst = hs[i].ap()[:, :, t0 : t0 + tcnt, :]
                else:
                    dst = hs[i].ap()[b, g * 128 : g * 128 + pg, t0 : t0 + tcnt, :]
                nc.sync.dma_start(dst, zsb)

            def finish_layer(i, zmax, nslots):
                zred = statp.tile([128, 1], f32, tag=f"zred{i}")
                nc.vector.tensor_reduce(
                    zred[:], zmax[:, :nslots], axis=AX.X, op=ALU.max
                )
                zall = statp.tile([128, 1], f32, tag=f"zall{i}")
                nc.gpsimd.partition_all_reduce(
                    zall[:], zred[:], 128, bass_isa.ReduceOp.max
                )
                ci, co = ccs[i]
                nc.sync.dma_start(ci.ap(), zall[:, 0])
                if not no_collective:
                    nc.gpsimd.collective_compute(
                        "AllReduce", ALU.max,
                        replica_groups=[list(range(N_CORES))],
                        ins=[ci.ap()], outs=[co.ap()],
                    )
                sraw = statp.tile([128, 1], f32, tag=f"sraw{i}")
                nc.sync.dma_start(sraw[:], (ci if no_collective else co).ap())
                sb = statp.tile([128, 1], f32, tag=f"sb{i}")
                nc.vector.tensor_scalar_max(sb[:], sraw[:], 1e-5)
                # exactly-rounded fl(127/s) via 2Prod + one Markstein step
                # (reference computes 127.0/s; a double-rounded 127*recip(s)
                # flips ~25% of quant multipliers by 1 ulp -> round() flips)
                def _tt(o, a, b, op):
                    nc.vector.tensor_tensor(o[:], a[:], b[:], op)

                def _tl(nm):
                    return statp.tile([128, 1], f32, tag=f"{nm}{i}", name=nm)

                r0 = _tl("r0")
                nc.vector.reciprocal(r0[:], sb[:])
                y0 = _tl("y0")
                nc.vector.tensor_scalar_mul(y0[:], r0[:], 127.0)
                VC = 4097.0  # Veltkamp split constant 2**12+1
                t1 = _tl("t1"); nc.vector.tensor_scalar_mul(t1[:], sb[:], VC)
                t2 = _tl("t2"); _tt(t2, t1, sb, ALU.subtract)
                ahi = _tl("ahi"); _tt(ahi, t1, t2, ALU.subtract)
                alo = _tl("alo"); _tt(alo, sb, ahi, ALU.subtract)
                u1 = _tl("u1"); nc.vector.tensor_scalar_mul(u1[:], y0[:], VC)
                u2 = _tl("u2"); _tt(u2, u1, y0, ALU.subtract)
                bhi = _tl("bhi"); _tt(bhi, u1, u2, ALU.subtract)
                blo = _tl("blo"); _tt(blo, y0, bhi, ALU.subtract)
                pp_ = _tl("pp_"); _tt(pp_, sb, y0, ALU.mult)
                e = _tl("e")
                _tt(e, ahi, bhi, ALU.mult)
                _tt(e, e, pp_, ALU.subtract)
                t3 = _tl("t3"); _tt(t3, ahi, blo, ALU.mult); _tt(e, e, t3, ALU.add)
                t4 = _tl("t4"); _tt(t4, alo, bhi, ALU.mult); _tt(e, e, t4, ALU.add)
                t5 = _tl("t5"); _tt(t5, alo, blo, ALU.mult); _tt(e, e, t5, ALU.add)
                rho = _tl("rho")
                nc.vector.tensor_scalar(rho[:], pp_[:], -127.0, -1.0, ALU.add, ALU.mult)
                _tt(rho, rho, e, ALU.subtract)
                cor = _tl("cor"); _tt(cor, rho, r0, ALU.mult)
                inv = statp.tile([128, 1], f32, tag=f"inv{i}")
                _tt(inv, y0, cor, ALU.add)
                state[("inv", i + 1)] = inv
                ds = statp.tile([128, GS[i + 1]], f32, tag=f"ds{i + 1}")
                nc.vector.tensor_scalar_mul(ds[:], cv[("sw", i + 1)][:], sb[:, 0:1])
                state[("ds", i + 1)] = ds

            def quantize(i, zin_ap, qdst_ap):
                inv = state[("inv", i)]
                pg = zin_ap.shape[0]
                nc.vector.tensor_scalar(
                    zin_ap, zin_ap, inv[:pg, 0:1], MAGIC, ALU.mult, ALU.add
                )
                nc.vector.tensor_scalar_add(qdst_ap, zin_ap, -MAGIC)

            # ================= layer 1 (im2col, contraction=5) =================
            zmax = statp.tile([128, 256], f32, tag="zmax")
            nc.gpsimd.memset(zmax[:], 0.0)
            slot = 0
            to1 = touts[0]
            with tc.tile_pool(name="imcol", bufs=5) as imcolp:
                for sg in range(_ceil_div(to1, L1_SEG)):
                    s0 = sg * L1_SEG
                    scnt = min(L1_SEG, to1 - s0)
                    wlen = 3 * (scnt - 1) + 1
                    imcs = []
                    for b in range(BPC):
                        imc = imcolp.tile(
                            [5, 3 * L1_SEG, 2], bf16, tag="imcol", name="imcol"
                        )
                        for k in range(5):
                            nc.sync.dma_start(
                                imc[k : k + 1, :wlen, :],
                                qx.ap()[b, 3 * s0 + k : 3 * s0 + k + wlen, :],
                            )
                        imcs.append(imc)
                    for tt0 in range(0, scnt, TT):
                        tcnt = min(TT, scnt - tt0)
                        ps = psum.tile([128, TT, 2], f32, tag="ps", name="ps")[:, :tcnt, :]
                        for b in range(BPC):
                            rhs = imcs[b][:, 3 * tt0 : 3 * (tt0 + tcnt - 1) + 1 : 3, :]
                            nc.tensor.matmul(
                                ps[32 * b : 32 * b + 32], w1s[:], rhs,
                                start=True, stop=True,
                                tile_position=(0, 32 * b),
                                skip_group_check=True,
                            )
                        post_process(0, ps, 128, 0, None, s0 + tt0, tcnt, zmax, slot)
                        slot += 1
            finish_layer(0, zmax, slot)

            # ================= layer 2 (Cin=32), in two time-halves =============
            zmax = statp.tile([128, 256], f32, tag="zmax")
            nc.gpsimd.memset(zmax[:], 0.0)
            slot = 0
            ti, to = tins[1], touts[1]
            for hh in range(2):
                h0 = hh * l2half
                h1 = min(to, h0 + l2half)
                tlo = 3 * h0 - 2            # logical input t of buffer col 0
                L = 3 * (h1 - 1) + 2 + 1 - tlo
                q1 = qbpool.tile([128, QL, 2], bf16, tag="qsb", name="q1h")[:, :L, :]
                lo_cl = max(tlo, 0)
                hi_cl = min(tlo + L, ti)
                if lo_cl > tlo:
                    nc.gpsimd.memset(q1[:, : lo_cl - tlo, :], 0.0)
                if hi_cl < tlo + L:
                    nc.gpsimd.memset(q1[:, hi_cl - tlo :, :], 0.0)
                for t0 in range(lo_cl, hi_cl, WQ):
                    w = min(WQ, hi_cl - t0)
                    zin = zinp.tile([128, WQ, 2], f32, tag="zin", name="zin")[:, :w, :]
                    nc.sync.dma_start(zin, hs[0].ap()[:, :, t0 : t0 + w, :])
                    quantize(1, zin, q1[:, t0 - tlo : t0 - tlo + w, :])
                for b in range(BPC):
                    q1b = qbpool.tile([32, QL, 2], bf16, tag="qsb", name="q1b")[:, :L, :]
                    nc.sync.dma_start(q1b, q1[32 * b : 32 * b + 32, :, :])
                    for tt0 in range(h0, h1, TT):
                        tcnt = min(TT, h1 - tt0)
                        ps = psum.tile([128, TT, 2], f32, tag="ps", name="ps")[:, :tcnt, :]
                        for k in range(5):
                            a = 3 * (tt0 - h0) + k  # 3*tt0+k-2 - tlo
                            rhs = q1b[:, a : a + 3 * (tcnt - 1) + 1 : 3, :]
                            nc.tensor.matmul(
                                ps, w2s[:, k, :], rhs, start=(k == 0), stop=(k == 4)
                            )
                        post_process(1, ps, 128, 0, b, tt0, tcnt, zmax, slot)
                        slot += 1
            finish_layer(1, zmax, slot)

            # ============ layers 3..6: Cin = 128*chunks ============
            for li in range(2, 6):
                cin, cout, K, stride, pp = LAYERS[li]
                chunks = _ceil_div(cin, 128)
                G = GS[li]
                ti, to = tins[li], touts[li]
                has_snake = li < 5
                if has_snake:
                    zmax = statp.tile([128, 256], f32, tag="zmax")
                    nc.gpsimd.memset(zmax[:], 0.0)
                slot = 0
                if li == 2:
                    pass  # w3s resident
                elif li == 3:
                    w4s = wbig.tile([128, 4, 5, 1024], bf16, tag="wbig")
                    nc.sync.dma_start(w4s[:], wts[3].ap())
                elif li == 4:
                    w5a = wbig.tile([128, 4, 5, 1024], bf16, tag="wbig")
                    nc.sync.dma_start(w5a[:], wts[4].ap()[:, 0:4])
                    w5b = wbig.tile([128, 4, 5, 1024], bf16, tag="wbig")
                    nc.sync.dma_start(w5b[:], wts[4].ap()[:, 4:8])
                for b in range(BPC):
                    qsb = qbpool.tile([128, QL, 2], bf16, tag="qsb", name="qsb")
                    qsb = qsb[:, : chunks * (ti + 2 * pp), :].rearrange(
                        "p (c t) two -> p c t two", c=chunks
                    )
                    nc.gpsimd.memset(qsb[:, :, 0:pp, :], 0.0)
                    nc.gpsimd.memset(qsb[:, :, ti + pp : ti + 2 * pp, :], 0.0)
                    for c in range(chunks):
                        for t0 in range(0, ti, WQ):
                            w = min(WQ, ti - t0)
                            zin = zinp.tile([128, WQ, 2], f32, tag="zin", name="zin")[:, :w, :]
                            nc.sync.dma_start(
                                zin,
                                hs[li - 1].ap()[
                                    b, c * 128 : c * 128 + 128, t0 : t0 + w, :
                                ],
                            )
                            quantize(li, zin, qsb[:, c, pp + t0 : pp + t0 + w, :])
                    for tt0 in range(0, to, TT):
                        tcnt = min(TT, to - tt0)
                        for g in range(G):
                            pg = min(128, cout - g * 128)
                            ps = psum.tile([128, TT, 2], f32, tag="ps", name="ps")[:pg, :tcnt, :]
                            nmm = chunks * K
                            mi = 0
                            for c in range(chunks):
                                for k in range(K):
                                    if li == 2:
                                        lwk = w3s[:, k, g * 128 : g * 128 + pg]
                                    elif li == 3:
                                        lwk = w4s[:, c, k, g * 128 : g * 128 + pg]
                                    elif li == 4:
                                        lwk = (w5a if c < 4 else w5b)[
                                            :, c % 4, k, g * 128 : g * 128 + pg
                                        ]
                                    else:
                                        lwk = wps[:, c, k, :]
                                    a = stride * tt0 + k
                                    rhs = qsb[
                                        :, c, a : a + stride * (tcnt - 1) + 1 : stride, :
                                    ]
                                    nc.tensor.matmul(
                                        ps, lwk, rhs,
                                        start=(mi == 0), stop=(mi == nmm - 1),
                                    )
                                    mi += 1
                            post_process(
                                li, ps, pg, g, b, tt0, tcnt,
                                zmax if has_snake else None, slot,
                            )
                            slot += 1
                if has_snake:
                    finish_layer(li, zmax, slot)
    return touts


def _host_prep(inputs, t1):
    """Replicate reference quantization bit-for-bit on jax-cpu."""
    import jax
    import jax.numpy as jnp

    cpu = jax.devices("cpu")[0]
    x = np.asarray(inputs["x"])
    ws = [inputs[f"w{i + 1}"] for i in range(5)] + [inputs["wp"]]
    bs = [inputs[f"b{i + 1}"] for i in range(5)] + [inputs["bp"]]
    las = [inputs[f"a{i + 1}"] for i in range(5)]
    lbs = [inputs[f"be{i + 1}"] for i in range(5)]

    with jax.default_device(cpu):
        xj = jnp.asarray(x.reshape(B, t1, 2))
        s1 = jnp.maximum(jnp.max(jnp.abs(xj)), 1e-5)
        q = jnp.clip(jnp.round(xj * (127.0 / s1)), -128.0, 127.0)
        qx = np.asarray(q, np.float32)
        s1 = float(np.asarray(s1, np.float32))
        terns, sws = [], []
        for w in ws:
            wj = jnp.asarray(np.asarray(w))
            sw = jnp.maximum(
                jnp.mean(jnp.abs(wj), axis=(1, 2, 3), keepdims=True), 1e-5
            )
            tern = jnp.clip(jnp.round(wj / sw), -1.0, 1.0)
            terns.append(np.asarray(tern, np.float32))
            sws.append(np.asarray(sw, np.float32).reshape(-1))

    qx_pad = np.zeros((B, t1 + 4, 2), np.float32)
    qx_pad[:, 2 : t1 + 2, :] = qx
    qx_pad = qx_pad.astype(ml_dtypes.bfloat16)

    def pack_vec(v, g):
        v = np.asarray(v, np.float32).reshape(-1)
        o = np.zeros((128, g), np.float32)
        o[np.arange(v.size) % 128, np.arange(v.size) // 128] = v
        return o

    feed = {}
    feed["w1"] = terns[0][:, 0, :, 0].T.astype(ml_dtypes.bfloat16)
    feed["w2"] = terns[1][:, :, :, 0].transpose(1, 2, 0).astype(ml_dtypes.bfloat16)
    feed["w3"] = terns[2][:, :, :, 0].transpose(1, 2, 0).astype(ml_dtypes.bfloat16)
    feed["w4"] = np.ascontiguousarray(
        terns[3][:, :, :, 0].transpose(1, 2, 0).reshape(4, 128, 5, 1024)
        .transpose(1, 0, 2, 3)
    ).astype(ml_dtypes.bfloat16)
    feed["w5"] = np.ascontiguousarray(
        terns[4][:, :, :, 0].transpose(1, 2, 0).reshape(8, 128, 5, 1024)
        .transpose(1, 0, 2, 3)
    ).astype(ml_dtypes.bfloat16)
    feed["w6"] = np.ascontiguousarray(
        terns[5][:, :, :, 0].transpose(1, 2, 0).reshape(8, 128, 3, 1)
        .transpose(1, 0, 2, 3)
    ).astype(ml_dtypes.bfloat16)

    for i in range(6):
        rep = (lambda v: np.tile(np.asarray(v, np.float32), BPC)) if i == 0 else (lambda v: v)
        feed[f"bias{i + 1}"] = pack_vec(rep(bs[i]), GS[i])
        if i < 5:
            feed[f"alpha{i + 1}"] = pack_vec(
                rep(np.exp(np.asarray(las[i], np.float32))), GS[i]
            )
            ib = 1.0 / (np.exp(np.asarray(lbs[i], np.float32)) + 1e-9)
            feed[f"sqib{i + 1}"] = pack_vec(rep(np.sqrt(ib).astype(np.float32)), GS[i])
    feed["ds1"] = pack_vec(np.tile((s1 / 127.0) * sws[0], BPC), 1)
    for i in range(1, 6):
        feed[f"sw{i + 1}"] = pack_vec(sws[i] / 127.0, GS[i])
    return feed, qx_pad


_CACHE = {}


def _get_compiled(t1):
    if t1 in _CACHE:
        return _CACHE[t1]
    from concourse import bacc
    from concourse.bass_interp import get_hw_module

    nc = bacc.Bacc(
        "TRN2", target_bir_lowering=False, debug=False,
        enable_asserts=True, num_devices=N_CORES,
    )
    _build(nc, t1)
    nc.compile()
    nc.m = get_hw_module(nc.m)
    _CACHE[t1] = nc
    return nc


def kernel(**inputs):
    from concourse import bass_utils

    t1 = inputs["x"].shape[2] // 2
    feed, qx_pad = _host_prep(inputs, t1)
    nc = _get_compiled(t1)

    in_maps = []
    for c in range(N_CORES):
        m = dict(feed)
        m["qx"] = np.ascontiguousarray(qx_pad[c * BPC : (c + 1) * BPC])
        in_maps.append(m)

    res = bass_utils.run_bass_kernel_spmd(nc, in_maps, core_ids=list(range(N_CORES)))
    outs = res.results
    hs = [
        np.concatenate([outs[c][f"h{i + 1}"] for c in range(N_CORES)], axis=0)
        for i in range(6)
    ]
    out = hs[5].reshape(B, -1)
    return (out, tuple(hs))
